# revision 1
# baseline (speedup 1.0000x reference)
"""Trainium2 Bass kernel for nn_DetectionLoss (YOLO-style detection loss).

Strategy (data-parallel over batch, 2 images per core x 8 cores):
  Dense part: obj BCE at non-positive cells reduces to sum(softplus(pred[...,4]))
    over the whole grid (the obj target is 0 there). Computed by streaming pred
    rows and reducing softplus of channel 4 on the Scalar engine (Exp + Ln(x+1)).
  Sparse part: at most B*N*A = 288 (cell,anchor) entries per core receive
    positive/box/cls loss. Rows are fetched with indirect DMA gathers and all
    assignment logic (anchor IoU, per-cell dedup via max-IoU, obj/cls targets)
    is computed with [96, 9]-shaped vector ops; cross-entry (same-cell)
    comparisons use a 32x32 block transpose + DRAM broadcast round trip.
  Final normalization (divide by num_pos etc.) happens on host after summing
    the 18 per-core scalar accumulators (the "all-reduce" of the sharding hint).
"""

import numpy as np

import concourse.bacc as bacc
import concourse.bass as bass
import concourse.tile as tile
from concourse import mybir
from concourse.bass_utils import run_bass_kernel_spmd

F32 = mybir.dt.float32
I32 = mybir.dt.int32
AF = mybir.ActivationFunctionType
OP = mybir.AluOpType
AX = mybir.AxisListType

# ---- problem constants (hardcoded per contract) ----
B, N, A, C = 16, 48, 3, 80
NCORES = 8
BLOC = B // NCORES          # 2 images per core
NP = BLOC * N               # 96 entry partitions
STRIDES = (8.0, 16.0, 32.0)
WS = (80, 40, 20)
HWS = (6400, 1600, 400)
RS = [BLOC * A * hw for hw in HWS]            # rows per scale per core
SBASE = [0, RS[0], RS[0] + RS[1]]             # scale row bases
ROWS = RS[0] + RS[1] + RS[2]                  # 50400
ROWS_PAD = 50688                              # 396 * 128, divisible by 3
NCOL = [300, 75, 21]                          # rows per partition per scale (s2 padded)
PAD_VAL = -60.0                               # softplus(PAD_VAL) == 0 in f32
EPS = 1e-7
# dense streaming chunks: (scale, col_start, width) in rows-per-partition units
CHUNKS = [(0, 0, 75), (0, 75, 75), (0, 150, 75), (0, 225, 75), (1, 0, 75), (2, 0, 21)]

# atan(z)/z poly in z^2 on [0,1], max abs err 5.8e-7
ATAN_C = [0.9999997152904466, -0.33327976036522494, 0.1989502583419013,
          -0.1353767514232845, 0.08475969773639125, -0.03775170756922951,
          0.008097294930236264]

_CACHE = {}
NUM_SWDGE_QUEUES = 1

# Pin exp/ln activations to the one table set containing both, so the
# compiler emits a single ACT_TABLE_LOAD instead of thrashing (~2.7us each).
# Positions in the list are preserved (they are the act_func_set ids).
_orig_get_act_tables = bacc.get_activation_tables


def _pinned_act_tables(arch):
    tables = _orig_get_act_tables(arch)
    keep = "natural_log_exp_and_others"
    if keep in tables:
        for name, funcs in tables.items():
            if name != keep:
                funcs.discard(AF.Exp)
                funcs.discard(AF.Ln)
    return tables


bacc.get_activation_tables = _pinned_act_tables

BATCH_GATHER = False
GATHER_OFF = False
GATHER_SPLIT = True
FULL_DEDUP = False
DEBUG_G = False


def _rap(ap, off_elems, pattern):
    """Raw AP at element offset relative to `ap`'s origin with [step,count] pairs.
    First pattern entry is the partition-dim pair."""
    return bass.AP(tensor=ap.tensor, offset=ap.offset + off_elems, ap=pattern)


def _flat(ap3):
    """[P, a, b] view -> [P, a*b]."""
    return ap3.rearrange("p a b -> p (a b)")


def build_nc(variant="v1", repeat=1):
    nc = bacc.Bacc(num_swdge_queues=NUM_SWDGE_QUEUES)
    rows = nc.dram_tensor("rows", [ROWS_PAD // 3, 255], F32, kind="ExternalInput")
    ch4 = nc.dram_tensor("ch4", [128, 396], F32, kind="ExternalInput")
    gt = nc.dram_tensor("gt", [NP, 4], F32, kind="ExternalInput")
    lbl = nc.dram_tensor("lbl", [NP, 1], F32, kind="ExternalInput")
    cc = nc.dram_tensor("cc", [1, 18], F32, kind="ExternalInput")
    anc0 = nc.dram_tensor("anc0", [3, 80, 80, 4], F32, kind="ExternalInput")
    anc1 = nc.dram_tensor("anc1", [3, 40, 40, 4], F32, kind="ExternalInput")
    anc2 = nc.dram_tensor("anc2", [3, 20, 20, 4], F32, kind="ExternalInput")
    out = nc.dram_tensor("out", [1, 18], F32, kind="ExternalOutput")
    dbg_g = nc.dram_tensor("dbg_g", [NP, 9 * 85], F32, kind="ExternalOutput") if DEBUG_G else None
    dbg_i = nc.dram_tensor("dbg_i", [NP, 3], I32, kind="ExternalOutput") if DEBUG_G else None

    with tile.TileContext(nc) as tc:
        for _rep in range(repeat):
            with tc.tile_pool(name=f"sing{_rep}", bufs=1) as sg, \
                 tc.tile_pool(name=f"dchunk{_rep}", bufs=3) as dpool, \
                 tc.tile_pool(name=f"dram{_rep}", bufs=1, space="DRAM") as drp, \
                 tc.tile_pool(name=f"psum{_rep}", bufs=1, space="PSUM") as psp:

                V = nc.vector

                # ---------------- loads ----------------
                ccb = sg.tile([NP, 6, 3], F32)      # const groups x scales
                cc0 = cc[:, :]
                nc.gpsimd.dma_start(out=ccb[:], in_=_rap(cc0, 0, [[0, NP], [3, 6], [1, 3]]))

                gtt = sg.tile([NP, 4], F32)
                nc.sync.dma_start(out=gtt[:], in_=gt[:, :])
                lblc = sg.tile([NP, 1], F32)
                nc.sync.dma_start(out=lblc[:], in_=lbl[:, :])

                ANC = sg.tile([NP, 3, 3, 4], F32)   # (s, a, xyxy) of cell (0,0)
                for s, anc in enumerate((anc0, anc1, anc2)):
                    a0 = anc[:, :, :, :]
                    nc.gpsimd.dma_start(
                        out=ANC[:, s, :, :],
                        in_=_rap(a0, 0, [[0, NP], [HWS[s] * 4, 3], [1, 4]]))

                def cg(g):  # [NP,3] const view, per scale
                    return ccb[:, g, :]

                def b9(col):  # [NP,1] -> [NP,9] free broadcast
                    return col.to_broadcast([NP, 9])

                def b3(col):
                    return col.to_broadcast([NP, 3])

                def r3(ap2d):  # [NP,9] -> [NP,3,3]
                    return ap2d.rearrange("p (s a) -> p s a", a=3)

                def mk9_from_s(src3):
                    """materialize [NP,9] tile broadcasting a per-scale [NP,3] over a"""
                    t = sg.tile([NP, 9], F32, tag=f"mk9_{nc.next_id()}")
                    src = bass.AP(tensor=src3.tensor, offset=src3.offset,
                                  ap=[src3.ap[0], src3.ap[1], [0, 3]])
                    V.tensor_copy(r3(t[:]), src)
                    return t

                # ---------------- dense: sum softplus(ch4) ----------------
                dsum = sg.tile([128, 3], F32)
                V.memset(dsum[:], 0.0)
                if variant == "v2":
                    c4t = sg.tile([128, 396], F32)
                    nc.sync.dma_start(out=c4t[:], in_=ch4[:, :])
                    cb = [0, 300, 375]
                    for s in range(3):
                        w = NCOL[s]
                        e = dpool.tile([128, 300], F32, tag="dexp")
                        nc.scalar.activation(out=e[:, :w], in_=c4t[:, cb[s]:cb[s] + w],
                                             func=AF.Exp)
                        sp = dpool.tile([128, 300], F32, tag="dsp")
                        nc.scalar.activation(out=sp[:, :w], in_=e[:, :w], func=AF.Ln,
                                             bias=1.0, accum_out=dsum[:, s:s + 1])
                else:
                    r0 = rows[:, :]
                    accs = []
                    for i, (s, c0, w) in enumerate(CHUNKS):
                        t = dpool.tile([128, 75, 85], F32, tag="dch")
                        nc.sync.dma_start(
                            out=t[:, :w, :],
                            in_=_rap(r0, (SBASE[s] + c0) * 85,
                                     [[NCOL[s] * 85, 128], [85, w], [1, 85]]))
                        e = dpool.tile([128, 75], F32, tag="dexp")
                        nc.scalar.activation(out=e[:, :w], in_=_flat(t[:, :w, 4:5]),
                                             func=AF.Exp)
                        sp = dpool.tile([128, 75], F32, tag="dsp")
                        acc = sg.tile([128, 1], F32, tag=f"dacc{i}")
                        nc.scalar.activation(out=sp[:, :w], in_=e[:, :w], func=AF.Ln,
                                             bias=1.0, accum_out=acc[:])
                        accs.append((s, acc))
                    for s, acc in accs:
                        V.tensor_add(dsum[:, s:s + 1], dsum[:, s:s + 1], acc[:])

                # ---------------- sparse: entry geometry ----------------
                x1, y1 = gtt[:, 0:1], gtt[:, 1:2]
                x2, y2 = gtt[:, 2:3], gtt[:, 3:4]
                gcx = sg.tile([NP, 1], F32)
                V.tensor_add(gcx[:], x1, x2)
                V.tensor_scalar_mul(gcx[:], gcx[:], 0.5)
                gcy = sg.tile([NP, 1], F32)
                V.tensor_add(gcy[:], y1, y2)
                V.tensor_scalar_mul(gcy[:], gcy[:], 0.5)

                def floor_clip(src, dst, tagp):
                    # dst[NP,3] = clip(trunc-toward-floor(src * inv_stride), 0, W-1)
                    V.tensor_mul(dst[:], b3(src[:]), cg(0))
                    ti = sg.tile([NP, 3], I32, tag=f"{tagp}_i")
                    V.tensor_copy(ti[:], dst[:])
                    tf = sg.tile([NP, 3], F32, tag=f"{tagp}_f")
                    V.tensor_copy(tf[:], ti[:])
                    adj = sg.tile([NP, 3], F32, tag=f"{tagp}_a")
                    V.tensor_tensor(out=adj[:], in0=tf[:], in1=dst[:], op=OP.is_gt)
                    V.tensor_sub(dst[:], tf[:], adj[:])
                    V.tensor_scalar_max(dst[:], dst[:], 0.0)
                    V.tensor_tensor(out=dst[:], in0=dst[:], in1=cg(2), op=OP.min)

                gx = sg.tile([NP, 3], F32)
                floor_clip(gcx, gx, "fcx")
                gy = sg.tile([NP, 3], F32)
                floor_clip(gcy, gy, "fcy")

                ck = sg.tile([NP, 3], F32)  # cell key per scale
                V.tensor_mul(ck[:], gy[:], cg(3))
                V.tensor_add(ck[:], ck[:], gx[:])

                # entry/partition index columns
                pidx = sg.tile([NP, 1], I32)
                nc.gpsimd.iota(pidx[:], pattern=[[0, 1]], base=0, channel_multiplier=1)
                pcol = sg.tile([NP, 1], F32)
                V.tensor_copy(pcol[:], pidx[:])
                bsel = sg.tile([NP, 1], F32)   # 1.0 for local image 1 (p >= 48)
                V.tensor_single_scalar(out=bsel[:], in_=pcol[:], scalar=47.5, op=OP.is_gt)

                stride9 = mk9_from_s(cg(1))

                # rows are ordered [b, cell, a] per scale; view them as triples
                # of 255 floats so one gather pulls an entry's 3 anchor rows.
                # triple index = base3_s + b*HW_s + cell
                idxf = sg.tile([NP, 3], F32)
                V.tensor_mul(idxf[:], b3(bsel[:]), cg(4))
                V.tensor_add(idxf[:], idxf[:], ck[:])
                V.tensor_add(idxf[:], idxf[:], cg(5))
                idx = sg.tile([NP, 3], I32)
                V.tensor_copy(idx[:], idxf[:])

                # ---------------- gathers ----------------
                # separate destination tiles so the 3 indirect DMAs pipeline
                # (slice-writes into one tile serialize on WAW tracking);
                # repack afterwards with cheap DVE copies.
                G = sg.tile([NP, 9, 85], F32)
                if GATHER_OFF:
                    V.memset(G[:], 0.1)
                else:
                    gks = []
                    for k in range(3):
                        # flat [NP, 255] dest: the SWDGE ucode scales indices by the
                        # dest's innermost contiguous run, which must be 255
                        gk = sg.tile([NP, 255], F32, tag=f"gk{k}")
                        nc.gpsimd.indirect_dma_start(
                            out=gk[:], out_offset=None, in_=rows[:, :],
                            in_offset=bass.IndirectOffsetOnAxis(ap=idx[:, k:k + 1], axis=0))
                        gks.append(gk)
                    for k in range(3):
                        V.tensor_copy(G[:, k * 3:(k + 1) * 3, :],
                                      gks[k][:].rearrange("p (a b) -> p a b", b=85))
                if DEBUG_G and _rep == 0:
                    nc.sync.dma_start(out=dbg_g[:, :], in_=G[:].rearrange("p a b -> p (a b)"))
                    nc.sync.dma_start(out=dbg_i[:, :], in_=idx[:])

                # ---------------- anchor boxes & IoU ----------------
                AW = sg.tile([NP, 9], F32)
                V.tensor_sub(r3(AW[:]), ANC[:, :, :, 2], ANC[:, :, :, 0])
                AH = sg.tile([NP, 9], F32)
                V.tensor_sub(r3(AH[:]), ANC[:, :, :, 3], ANC[:, :, :, 1])
                AWH = sg.tile([NP, 9], F32)
                V.tensor_scalar_mul(AWH[:], AW[:], 0.5)
                AHH = sg.tile([NP, 9], F32)
                V.tensor_scalar_mul(AHH[:], AH[:], 0.5)

                acx = sg.tile([NP, 3], F32)
                V.tensor_scalar_add(acx[:], gx[:], 0.5)
                V.tensor_mul(acx[:], acx[:], cg(1))
                acy = sg.tile([NP, 3], F32)
                V.tensor_scalar_add(acy[:], gy[:], 0.5)
                V.tensor_mul(acy[:], acy[:], cg(1))
                acx9 = mk9_from_s(acx[:])
                acy9 = mk9_from_s(acy[:])

                ax1 = sg.tile([NP, 9], F32)
                V.tensor_sub(ax1[:], acx9[:], AWH[:])
                ax2 = sg.tile([NP, 9], F32)
                V.tensor_add(ax2[:], acx9[:], AWH[:])
                ay1 = sg.tile([NP, 9], F32)
                V.tensor_sub(ay1[:], acy9[:], AHH[:])
                ay2 = sg.tile([NP, 9], F32)
                V.tensor_add(ay2[:], acy9[:], AHH[:])

                ag = sg.tile([NP, 1], F32)
                w2 = sg.tile([NP, 1], F32)
                h2 = sg.tile([NP, 1], F32)
                V.tensor_sub(w2[:], x2, x1)
                V.tensor_sub(h2[:], y2, y1)
                V.tensor_mul(ag[:], w2[:], h2[:])

                aarea = sg.tile([NP, 9], F32)
                V.tensor_mul(aarea[:], AW[:], AH[:])

                # IoU(gt, anchor_box) = inter / (area_gt + area_anchor - inter + eps)
                t1 = sg.tile([NP, 9], F32)
                t2 = sg.tile([NP, 9], F32)
                iw0 = sg.tile([NP, 9], F32)
                V.tensor_tensor(out=t1[:], in0=ax1[:], in1=b9(x1), op=OP.max)
                V.tensor_tensor(out=t2[:], in0=ax2[:], in1=b9(x2), op=OP.min)
                V.tensor_sub(iw0[:], t2[:], t1[:])
                V.tensor_scalar_max(iw0[:], iw0[:], 0.0)
                ih0 = sg.tile([NP, 9], F32)
                V.tensor_tensor(out=t1[:], in0=ay1[:], in1=b9(y1), op=OP.max)
                V.tensor_tensor(out=t2[:], in0=ay2[:], in1=b9(y2), op=OP.min)
                V.tensor_sub(ih0[:], t2[:], t1[:])
                V.tensor_scalar_max(ih0[:], ih0[:], 0.0)
                inter0 = sg.tile([NP, 9], F32)
                V.tensor_mul(inter0[:], iw0[:], ih0[:])
                un0 = sg.tile([NP, 9], F32)
                V.tensor_add(un0[:], b9(ag[:]), aarea[:])
                V.tensor_sub(un0[:], un0[:], inter0[:])
                V.tensor_scalar_add(un0[:], un0[:], EPS)
                V.reciprocal(un0[:], un0[:])
                iou = sg.tile([NP, 9], F32)
                V.tensor_mul(iou[:], inter0[:], un0[:])

                # pos / best-anchor fallback
                pos0 = sg.tile([NP, 9], F32)
                V.tensor_single_scalar(out=pos0[:], in_=iou[:], scalar=0.5, op=OP.is_gt)

                def sa(t, a):  # [NP,3] strided per-anchor view of a [NP,9] tile
                    return bass.AP(tensor=t.tensor, offset=t.offset + a,
                                   ap=[t.ap[0], [3, 3]])

                i0, i1, i2 = sa(iou[:], 0), sa(iou[:], 1), sa(iou[:], 2)
                ge01 = sg.tile([NP, 3], F32)
                V.tensor_tensor(out=ge01[:], in0=i0, in1=i1, op=OP.is_ge)
                ge02 = sg.tile([NP, 3], F32)
                V.tensor_tensor(out=ge02[:], in0=i0, in1=i2, op=OP.is_ge)
                ge12 = sg.tile([NP, 3], F32)
                V.tensor_tensor(out=ge12[:], in0=i1, in1=i2, op=OP.is_ge)
                best = sg.tile([NP, 9], F32)
                bb0, bb1, bb2 = sa(best[:], 0), sa(best[:], 1), sa(best[:], 2)
                V.tensor_mul(bb0, ge01[:], ge02[:])
                t3 = sg.tile([NP, 3], F32)
                V.tensor_scalar(out=t3[:], in0=ge01[:], scalar1=-1.0, scalar2=1.0,
                                op0=OP.mult, op1=OP.add)
                V.tensor_mul(bb1, t3[:], ge12[:])
                V.tensor_scalar(out=t3[:], in0=bb0, scalar1=-1.0, scalar2=1.0,
                                op0=OP.mult, op1=OP.add)
                V.tensor_sub(bb2, t3[:], bb1)

                anyp = sg.tile([NP, 3], F32)
                V.tensor_reduce(out=anyp[:], in_=r3(pos0[:]), axis=AX.X, op=OP.max)
                nanyp = sg.tile([NP, 3], F32)
                V.tensor_scalar(out=nanyp[:], in0=anyp[:], scalar1=-1.0, scalar2=1.0,
                                op0=OP.mult, op1=OP.add)
                anyp9 = mk9_from_s(anyp[:])
                nanyp9 = mk9_from_s(nanyp[:])
                posf = sg.tile([NP, 9], F32)
                V.tensor_mul(posf[:], pos0[:], anyp9[:])
                tb = sg.tile([NP, 9], F32)
                V.tensor_mul(tb[:], best[:], nanyp9[:])
                V.tensor_add(posf[:], posf[:], tb[:])

                # ---------------- decode + CIoU ----------------
                sig = sg.tile([NP, 9, 4], F32)
                nc.scalar.activation(out=sig[:], in_=G[:, :, 0:4], func=AF.Exp, scale=-1.0)
                V.tensor_scalar_add(_flat(sig[:]), _flat(sig[:]), 1.0)
                V.reciprocal(_flat(sig[:]), _flat(sig[:]))

                def sigc(i):  # [NP,9] view of sigmoid column i
                    return _flat(sig[:, :, i:i + 1])

                pcx = sg.tile([NP, 9], F32)
                V.tensor_scalar(out=pcx[:], in0=sigc(0), scalar1=2.0, scalar2=-1.0,
                                op0=OP.mult, op1=OP.add)
                V.tensor_mul(pcx[:], pcx[:], stride9[:])
                V.tensor_add(pcx[:], pcx[:], acx9[:])
                pcy = sg.tile([NP, 9], F32)
                V.tensor_scalar(out=pcy[:], in0=sigc(1), scalar1=2.0, scalar2=-1.0,
                                op0=OP.mult, op1=OP.add)
                V.tensor_mul(pcy[:], pcy[:], stride9[:])
                V.tensor_add(pcy[:], pcy[:], acy9[:])

                pw = sg.tile([NP, 9], F32)
                V.tensor_scalar_mul(pw[:], sigc(2), 2.0)
                V.tensor_mul(pw[:], pw[:], pw[:])
                V.tensor_mul(pw[:], pw[:], AW[:])
                ph = sg.tile([NP, 9], F32)
                V.tensor_scalar_mul(ph[:], sigc(3), 2.0)
                V.tensor_mul(ph[:], ph[:], ph[:])
                V.tensor_mul(ph[:], ph[:], AH[:])

                pwh = sg.tile([NP, 9], F32)
                V.tensor_scalar_mul(pwh[:], pw[:], 0.5)
                px1 = sg.tile([NP, 9], F32)
                V.tensor_sub(px1[:], pcx[:], pwh[:])
                px2 = sg.tile([NP, 9], F32)
                V.tensor_add(px2[:], pcx[:], pwh[:])
                V.tensor_scalar_mul(pwh[:], ph[:], 0.5)
                py1 = sg.tile([NP, 9], F32)
                V.tensor_sub(py1[:], pcy[:], pwh[:])
                py2 = sg.tile([NP, 9], F32)
                V.tensor_add(py2[:], pcy[:], pwh[:])

                w1 = sg.tile([NP, 9], F32)
                V.tensor_sub(w1[:], px2[:], px1[:])
                h1 = sg.tile([NP, 9], F32)
                V.tensor_sub(h1[:], py2[:], py1[:])
                w2h2 = sg.tile([NP, 1], F32)
                V.tensor_mul(w2h2[:], w2[:], h2[:])

                # overlap with gt
                V.tensor_tensor(out=t1[:], in0=px1[:], in1=b9(x1), op=OP.max)
                V.tensor_tensor(out=t2[:], in0=px2[:], in1=b9(x2), op=OP.min)
                iw = sg.tile([NP, 9], F32)
                V.tensor_sub(iw[:], t2[:], t1[:])
                V.tensor_scalar_max(iw[:], iw[:], 0.0)
                V.tensor_tensor(out=t1[:], in0=py1[:], in1=b9(y1), op=OP.max)
                V.tensor_tensor(out=t2[:], in0=py2[:], in1=b9(y2), op=OP.min)
                ih = sg.tile([NP, 9], F32)
                V.tensor_sub(ih[:], t2[:], t1[:])
                V.tensor_scalar_max(ih[:], ih[:], 0.0)
                inter = sg.tile([NP, 9], F32)
                V.tensor_mul(inter[:], iw[:], ih[:])
                un = sg.tile([NP, 9], F32)
                V.tensor_mul(un[:], w1[:], h1[:])
                V.tensor_add(un[:], un[:], b9(w2h2[:]))
                V.tensor_sub(un[:], un[:], inter[:])
                V.tensor_scalar_add(un[:], un[:], EPS)
                V.reciprocal(un[:], un[:])
                iou2 = sg.tile([NP, 9], F32)
                V.tensor_mul(iou2[:], inter[:], un[:])

                cw = sg.tile([NP, 9], F32)
                V.tensor_tensor(out=t1[:], in0=px2[:], in1=b9(x2), op=OP.max)
                V.tensor_tensor(out=t2[:], in0=px1[:], in1=b9(x1), op=OP.min)
                V.tensor_sub(cw[:], t1[:], t2[:])
                chh = sg.tile([NP, 9], F32)
                V.tensor_tensor(out=t1[:], in0=py2[:], in1=b9(y2), op=OP.max)
                V.tensor_tensor(out=t2[:], in0=py1[:], in1=b9(y1), op=OP.min)
                V.tensor_sub(chh[:], t1[:], t2[:])
                c2t = sg.tile([NP, 9], F32)
                V.tensor_mul(c2t[:], cw[:], cw[:])
                V.tensor_mul(t1[:], chh[:], chh[:])
                V.tensor_add(c2t[:], c2t[:], t1[:])
                V.tensor_scalar_add(c2t[:], c2t[:], EPS)

                gx12 = sg.tile([NP, 1], F32)
                V.tensor_add(gx12[:], x1, x2)
                gy12 = sg.tile([NP, 1], F32)
                V.tensor_add(gy12[:], y1, y2)
                rho = sg.tile([NP, 9], F32)
                V.tensor_sub(rho[:], b9(gx12[:]), px1[:])
                V.tensor_sub(rho[:], rho[:], px2[:])
                V.tensor_mul(rho[:], rho[:], rho[:])
                rhoy = sg.tile([NP, 9], F32)
                V.tensor_sub(rhoy[:], b9(gy12[:]), py1[:])
                V.tensor_sub(rhoy[:], rhoy[:], py2[:])
                V.tensor_mul(rhoy[:], rhoy[:], rhoy[:])
                V.tensor_add(rho[:], rho[:], rhoy[:])
                V.tensor_scalar_mul(rho[:], rho[:], 0.25)

                # v term: atan(r2) - atan(r1) == atan((r2-r1)/(1+r1*r2)) for r1,r2>0
                r2c = sg.tile([NP, 1], F32)
                V.tensor_scalar_add(r2c[:], h2[:], EPS)
                V.reciprocal(r2c[:], r2c[:])
                V.tensor_mul(r2c[:], r2c[:], w2[:])
                r1t = sg.tile([NP, 9], F32)
                V.tensor_scalar_add(r1t[:], h1[:], EPS)
                V.reciprocal(r1t[:], r1t[:])
                V.tensor_mul(r1t[:], r1t[:], w1[:])
                num = sg.tile([NP, 9], F32)
                V.tensor_sub(num[:], b9(r2c[:]), r1t[:])
                den = sg.tile([NP, 9], F32)
                V.tensor_mul(den[:], b9(r2c[:]), r1t[:])
                V.tensor_scalar_add(den[:], den[:], 1.0)
                V.reciprocal(den[:], den[:])
                uu = sg.tile([NP, 9], F32)
                V.tensor_mul(uu[:], num[:], den[:])

                au = sg.tile([NP, 9], F32)
                V.tensor_scalar_mul(au[:], uu[:], -1.0)
                V.tensor_tensor(out=au[:], in0=au[:], in1=uu[:], op=OP.max)
                rau = sg.tile([NP, 9], F32)
                V.tensor_scalar_max(rau[:], au[:], 1e-30)
                V.reciprocal(rau[:], rau[:])
                zz = sg.tile([NP, 9], F32)
                V.tensor_tensor(out=zz[:], in0=au[:], in1=rau[:], op=OP.min)
                zq = sg.tile([NP, 9], F32)
                V.tensor_mul(zq[:], zz[:], zz[:])
                poly = sg.tile([NP, 9], F32)
                V.memset(poly[:], ATAN_C[-1])
                for coef in ATAN_C[-2::-1]:
                    V.tensor_mul(poly[:], poly[:], zq[:])
                    V.tensor_scalar_add(poly[:], poly[:], coef)
                V.tensor_mul(poly[:], poly[:], zz[:])
                gt1 = sg.tile([NP, 9], F32)
                V.tensor_single_scalar(out=gt1[:], in_=au[:], scalar=1.0, op=OP.is_gt)
                pm = sg.tile([NP, 9], F32)
                V.tensor_scalar(out=pm[:], in0=poly[:], scalar1=-1.0,
                                scalar2=float(np.pi / 2), op0=OP.mult, op1=OP.add)
                V.tensor_sub(pm[:], pm[:], poly[:])
                V.tensor_mul(pm[:], pm[:], gt1[:])
                at = sg.tile([NP, 9], F32)
                V.tensor_add(at[:], poly[:], pm[:])
                sgn = sg.tile([NP, 9], F32)
                V.tensor_single_scalar(out=sgn[:], in_=uu[:], scalar=0.0, op=OP.is_lt)
                V.tensor_scalar(out=sgn[:], in0=sgn[:], scalar1=-2.0, scalar2=1.0,
                                op0=OP.mult, op1=OP.add)
                V.tensor_mul(at[:], at[:], sgn[:])
                vv = sg.tile([NP, 9], F32)
                V.tensor_mul(vv[:], at[:], at[:])
                V.tensor_scalar_mul(vv[:], vv[:], float(4.0 / (np.pi ** 2)))

                alph = sg.tile([NP, 9], F32)
                V.tensor_sub(alph[:], vv[:], iou2[:])
                V.tensor_scalar(out=alph[:], in0=alph[:], scalar1=1.0, scalar2=EPS,
                                op0=OP.add, op1=OP.add)
                V.reciprocal(alph[:], alph[:])
                V.tensor_mul(alph[:], alph[:], vv[:])    # alpha
                V.tensor_mul(alph[:], alph[:], vv[:])    # v * alpha

                ciou = sg.tile([NP, 9], F32)
                V.reciprocal(c2t[:], c2t[:])
                V.tensor_mul(c2t[:], c2t[:], rho[:])     # rho2 / c2
                V.tensor_add(c2t[:], c2t[:], alph[:])
                V.tensor_sub(ciou[:], iou2[:], c2t[:])

                ciout = sg.tile([NP, 9], F32)
                V.tensor_scalar_max(ciout[:], ciou[:], 0.0)
                V.tensor_scalar_min(ciout[:], ciout[:], 1.0)

                # ---------------- transpose round trip ----------------
                pack = sg.tile([NP, 32], F32)
                negones = sg.tile([NP, 1], F32)
                V.memset(negones[:], -1.0)
                V.tensor_copy(pack[:, 0:9], iou[:])
                notpos = sg.tile([NP, 9], I32)
                V.tensor_single_scalar(out=notpos[:], in_=posf[:], scalar=0.5, op=OP.is_lt)
                V.copy_predicated(pack[:, 0:9], notpos[:], negones[:].to_broadcast([NP, 9]))
                if FULL_DEDUP:
                    V.tensor_copy(pack[:, 9:18], ciout[:])
                    V.tensor_copy(pack[:, 18:21], ck[:])
                    V.tensor_copy(pack[:, 21:22], lblc[:])
                    V.tensor_copy(pack[:, 22:23], bsel[:])
                    V.memset(pack[:, 23:32], 0.0)
                    NRB = 23
                    CKR, LBR, BSR = 18, 21, 22
                else:
                    V.tensor_copy(pack[:, 9:12], ck[:])
                    V.tensor_copy(pack[:, 12:13], bsel[:])
                    V.memset(pack[:, 13:32], 0.0)
                    NRB = 13
                    CKR, LBR, BSR = 9, 21, 12

                T = sg.tile([32, NP], F32)
                for blk in range(3):
                    V.transpose(out=T[:, blk * 32:(blk + 1) * 32],
                                in_=pack[blk * 32:(blk + 1) * 32, :])
                dsc = drp.tile([32, NP], F32)
                nc.sync.dma_start(out=dsc[:], in_=T[:])
                RB = sg.tile([NP, NRB, NP], F32, tag="RB")
                d0 = dsc[:, :]
                nc.gpsimd.dma_start(out=RB[:], in_=_rap(d0, 0, [[0, NP], [NP, NRB], [1, NP]]))

                def rbrow(r):  # [NP, NP] view of transposed row r
                    return RB[:, r:r + 1, :].rearrange("p o n -> p (o n)")

                # ---------------- same-cell logic (full 96-wide, batch mask folded) ---
                beq = sg.tile([NP, NP], F32)   # same local image
                V.tensor_scalar(out=beq[:], in0=rbrow(BSR), scalar1=bsel[:, :],
                                scalar2=None, op0=OP.is_equal)
                sm3 = sg.tile([NP, 3, NP], F32)
                for s in range(3):
                    ksl = sm3[:, s:s + 1, :].rearrange("p o n -> p (o n)")
                    V.tensor_scalar(out=ksl, in0=rbrow(CKR + s), scalar1=ck[:, s:s + 1],
                                    scalar2=None, op0=OP.is_equal)
                    V.tensor_mul(ksl, ksl, beq[:])
                same9 = sg.tile([NP, 9, NP], F32)   # broadcast over a
                s0 = sm3[:, :, :]
                sm4 = bass.AP(tensor=s0.tensor, offset=s0.offset,
                              ap=[s0.ap[0], [NP, 3], [0, 3], [1, NP]])
                V.tensor_copy(same9[:].rearrange("p (s a) n -> p s a n", a=3), sm4)

                nots9 = sg.tile([NP, 9, NP], I32)
                V.tensor_single_scalar(out=nots9[:], in_=same9[:], scalar=0.5, op=OP.is_lt)
                negt = sg.tile([NP, 9, NP], F32)
                V.memset(negt[:], -1.0)

                mv = sg.tile([NP, 9, NP], F32)
                V.tensor_copy(mv[:], RB[:, 0:9, :])
                V.copy_predicated(mv[:], nots9[:], negt[:])

                cellmax = sg.tile([NP, 9], F32)
                V.tensor_reduce(out=cellmax[:], in_=mv[:], axis=AX.X, op=OP.max)

                win = sg.tile([NP, 9], F32)
                V.tensor_tensor(out=win[:], in0=iou[:], in1=cellmax[:], op=OP.is_equal)
                V.tensor_mul(win[:], win[:], posf[:])

                if FULL_DEDUP:
                    wmask = sg.tile([NP, 9, NP], F32)
                if FULL_DEDUP:
                    cm = cellmax[:]
                    cmb = bass.AP(tensor=cm.tensor, offset=cm.offset,
                                  ap=[cm.ap[0], [1, 9], [0, NP]])
                    V.tensor_tensor(out=wmask[:], in0=mv[:], in1=cmb, op=OP.is_equal)

                    objt = sg.tile([NP, 9], F32)
                    wct = sg.tile([NP, 9, NP], F32)
                    V.tensor_mul(wct[:], wmask[:], RB[:, 9:18, :])
                    V.tensor_reduce(out=objt[:], in_=wct[:], axis=AX.X, op=OP.max)

                    # ltm[p, n'] = 1 if n' < p  (global entry order)
                    jrow_i = sg.tile([NP, NP], I32)
                    nc.gpsimd.iota(jrow_i[:], pattern=[[1, NP]], base=0, channel_multiplier=0)
                    jrow = sg.tile([NP, NP], F32)
                    V.tensor_copy(jrow[:], jrow_i[:])
                    ltm = sg.tile([NP, NP], F32)
                    V.tensor_scalar(out=ltm[:], in0=jrow[:], scalar1=pcol[:, :], scalar2=None,
                                    op0=OP.is_lt)
                    lt = ltm[:]
                    ltb = bass.AP(tensor=lt.tensor, offset=lt.offset,
                                  ap=[lt.ap[0], [0, 9], [1, NP]])
                    wl = sg.tile([NP, 9, NP], F32)
                    V.tensor_mul(wl[:], wmask[:], ltb)
                    excl = sg.tile([NP, 9], F32)
                    V.tensor_reduce(out=excl[:], in_=wl[:], axis=AX.X, op=OP.max)
                    rep = sg.tile([NP, 9], F32)
                    V.tensor_scalar(out=rep[:], in0=excl[:], scalar1=-1.0, scalar2=1.0,
                                    op0=OP.mult, op1=OP.add)
                    V.tensor_mul(rep[:], rep[:], win[:])

                    leq = sg.tile([NP, NP], F32)
                    V.tensor_scalar(out=leq[:], in0=rbrow(21), scalar1=lblc[:, :],
                                    scalar2=None, op0=OP.is_equal)
                    lq = leq[:]
                    lqb = bass.AP(tensor=lq.tensor, offset=lq.offset,
                                  ap=[lq.ap[0], [0, 9], [1, NP]])
                    V.tensor_mul(wl[:], wl[:], lqb)
                    exclc = sg.tile([NP, 9], F32)
                    V.tensor_reduce(out=exclc[:], in_=wl[:], axis=AX.X, op=OP.max)
                    repcl = sg.tile([NP, 9], F32)
                    V.tensor_scalar(out=repcl[:], in0=exclc[:], scalar1=-1.0, scalar2=1.0,
                                    op0=OP.mult, op1=OP.add)
                    V.tensor_mul(repcl[:], repcl[:], win[:])
                else:
                    # no bitwise-IoU ties => exactly one winner per cell:
                    # rep == repcl == win, obj target == own clipped ciou
                    rep = win
                    repcl = win
                    objt = ciout

                # ---------------- per-entry loss pieces ----------------
                p4v = _flat(G[:, :, 4:5])
                e4 = sg.tile([NP, 9], F32)
                nc.scalar.activation(out=e4[:], in_=p4v, func=AF.Exp)
                sp4 = sg.tile([NP, 9], F32)
                nc.scalar.activation(out=sp4[:], in_=e4[:], func=AF.Ln, bias=1.0)

                EC = sg.tile([NP, 9, 80], F32)
                nc.scalar.activation(out=EC[:], in_=G[:, :, 5:85], func=AF.Exp)
                nc.scalar.activation(out=EC[:], in_=EC[:], func=AF.Ln, bias=1.0)
                rs9 = sg.tile([NP, 9], F32)
                V.tensor_reduce(out=rs9[:], in_=EC[:], axis=AX.X, op=OP.add)

                ohi = sg.tile([NP, 80], I32)
                nc.gpsimd.iota(ohi[:], pattern=[[1, 80]], base=0, channel_multiplier=0)
                oh = sg.tile([NP, 80], F32)
                V.tensor_copy(oh[:], ohi[:])
                V.tensor_scalar(out=oh[:], in0=oh[:], scalar1=lblc[:, :], scalar2=None,
                                op0=OP.is_equal)
                og = oh[:]
                ohb = bass.AP(tensor=og.tensor, offset=og.offset,
                              ap=[og.ap[0], [0, 9], [1, 80]])
                PL = sg.tile([NP, 9, 80], F32)
                V.tensor_mul(PL[:], G[:, :, 5:85], ohb)
                pl9 = sg.tile([NP, 9], F32)
                V.tensor_reduce(out=pl9[:], in_=PL[:], axis=AX.X, op=OP.add)

                # ---------------- accumulate to 18 outputs ----------------
                pack18 = sg.tile([128, 18], F32)
                V.memset(pack18[96:128, 0:15], 0.0)

                def col3(q):  # strided [NP,3] view of pack18 cols {q, q+5, q+10}
                    sl = pack18[0:96, :]
                    return bass.AP(tensor=sl.tensor, offset=sl.offset + q,
                                   ap=[sl.ap[0], [5, 3]])

                def red3(src_ap, q):
                    V.tensor_reduce(out=col3(q), in_=r3(src_ap), axis=AX.X, op=OP.add)

                tacc = sg.tile([NP, 9], F32)
                V.tensor_scalar(out=tacc[:], in0=ciou[:], scalar1=-1.0, scalar2=1.0,
                                op0=OP.mult, op1=OP.add)
                V.tensor_mul(tacc[:], tacc[:], win[:])
                red3(tacc[:], 0)

                t4 = sg.tile([NP, 9], F32)
                V.tensor_copy(t4[:], p4v)
                V.tensor_mul(t4[:], t4[:], objt[:])
                V.tensor_sub(t4[:], sp4[:], t4[:])
                V.tensor_mul(t4[:], t4[:], rep[:])
                red3(t4[:], 1)

                V.tensor_mul(tacc[:], rep[:], rs9[:])
                t5 = sg.tile([NP, 9], F32)
                V.tensor_mul(t5[:], repcl[:], pl9[:])
                V.tensor_sub(tacc[:], tacc[:], t5[:])
                red3(tacc[:], 2)

                V.tensor_mul(tacc[:], rep[:], sp4[:])
                red3(tacc[:], 3)

                red3(rep[:], 4)

                for s in range(3):
                    V.tensor_copy(pack18[:, 15 + s:16 + s], dsum[:, s:s + 1])

                ones = sg.tile([128, 1], F32)
                V.memset(ones[:], 1.0)
                red_ps = psp.tile([128, 18], F32)
                nc.tensor.matmul(red_ps[:1], ones[:], pack18[:], start=True, stop=True)
                osb = sg.tile([1, 18], F32)
                V.tensor_copy(osb[:], red_ps[:1])
                nc.gpsimd.dma_start(out=out[:, :], in_=osb[:])

    nc.finalize()
    return nc


def _prep_core_inputs(inputs, core):
    """Slice/layout (no arithmetic) the full inputs for one core."""
    b0 = core * BLOC
    # rows ordered [b, cell, a] per scale so one entry's 3 anchor rows are
    # consecutive (single indirect gather per scale)
    preds = [np.asarray(inputs[f"pred{s}"][b0:b0 + BLOC], dtype=np.float32)
             .reshape(BLOC, A, HWS[s], 85).transpose(0, 2, 1, 3)
             for s in range(3)]
    rows = np.full((ROWS_PAD, 85), PAD_VAL, dtype=np.float32)
    off = 0
    for s in range(3):
        r = preds[s].reshape(-1, 85)
        rows[off:off + r.shape[0]] = r
        off += r.shape[0]
    # planar ch4, per-scale [128, ncol] blocks (same row order as `rows`)
    ch4 = np.empty((128, 396), np.float32)
    cb = [0, 300, 375]
    for s in range(3):
        plane = np.full(128 * NCOL[s], PAD_VAL, np.float32)
        pr = preds[s].reshape(-1, 85)[:, 4]
        plane[:pr.shape[0]] = pr
        ch4[:, cb[s]:cb[s] + NCOL[s]] = plane.reshape(128, NCOL[s])
    gtb = np.ascontiguousarray(
        inputs["gt_boxes"][b0:b0 + BLOC], dtype=np.float32).reshape(NP, 4)
    lblv = np.ascontiguousarray(
        inputs["gt_labels"][b0:b0 + BLOC]).astype(np.float32).reshape(NP, 1)
    cc = np.zeros((1, 18), np.float32)
    for s in range(3):
        cc[0, 0 + s] = 1.0 / STRIDES[s]
        cc[0, 3 + s] = STRIDES[s]
        cc[0, 6 + s] = WS[s] - 1
        cc[0, 9 + s] = WS[s]
        cc[0, 12 + s] = HWS[s]
        cc[0, 15 + s] = SBASE[s] // 3
    return {
        "rows": rows.reshape(ROWS_PAD // 3, 255), "ch4": ch4, "gt": gtb,
        "lbl": lblv, "cc": cc,
        "anc0": np.ascontiguousarray(inputs["anchors0"], dtype=np.float32),
        "anc1": np.ascontiguousarray(inputs["anchors1"], dtype=np.float32),
        "anc2": np.ascontiguousarray(inputs["anchors2"], dtype=np.float32),
    }


def _combine(parts):
    """Host-side all-reduce of the 18 per-core accumulators + final normalization."""
    acc = np.zeros(18, dtype=np.float64)
    for p in parts:
        acc += p.astype(np.float64)
    box_s = objp_s = cls_s = npos = 0.0
    objn_s = 0.0
    for s in range(3):
        box_s += acc[s * 5 + 0]
        objp_s += acc[s * 5 + 1]
        cls_s += acc[s * 5 + 2]
        negc = acc[s * 5 + 3]
        npos_s = acc[s * 5 + 4]
        dsum_s = acc[15 + s]
        npos += npos_s
        flat = B * A * HWS[s]
        num_neg = flat - npos_s
        objn_s += (dsum_s - negc) / max(num_neg, 1.0)
    tp = max(npos, 1.0)
    box_loss = box_s / tp
    obj_pos_loss = objp_s / tp
    obj_neg_loss = objn_s / 3.0
    cls_loss = cls_s / tp
    total = box_loss + obj_pos_loss + obj_neg_loss + cls_loss
    vals = [total, box_loss, obj_pos_loss, obj_neg_loss, cls_loss]
    if not np.isfinite(total):
        vals = [0.0] * 5
    return tuple(np.asarray(v, dtype=np.float32) for v in vals)


def kernel(**inputs):
    variant = inputs.pop("_variant", "v1")
    trace = inputs.pop("_trace", False)
    if variant not in _CACHE:
        _CACHE[variant] = build_nc(variant)
    nc = _CACHE[variant]
    in_maps = [_prep_core_inputs(inputs, c) for c in range(NCORES)]
    res = run_bass_kernel_spmd(nc, in_maps, core_ids=list(range(NCORES)), trace=trace)
    parts = [r["out"][0] for r in res.results]
    outv = _combine(parts)
    kernel._last_results = res
    return outv

